# revision 5
# baseline (speedup 1.0000x reference)
"""Trainium2 Bass kernel v2 for batched differentiable-MPC (LQR) controller.

Riccati backward sweep + forward rollout, B=512 data-parallel over 8 cores
(64 batches/core).

v2 layout ("col-major GRM"): per core, local batch b = 16*g + s with
partition-group g in [0,4) and slot s in [0,16). A per-batch 32x32 z-space
matrix M_b and augmented column q_b live in a [128, 528] fp16 tile:
    tile[32*g + i, 16*c + s] = M_b[i, c]   (c < 32)
    tile[32*g + i, 512 + s]  = q_b[i]      (aug col c=32)
Col-major slots make every column view contiguous and keep the DVE 2x fp16
mode on the broadcast FMA (broadcast over the middle dim, packed last dim).

Backward step: S = Z^T V (PE, fp16) -> J = per-block transpose (DVE, f32
PSUM->SBUF) -> Q = C + Z^T J, qz = q + Z^T v (PE accumulate) -> Act copies
PSUM->SBUF fp16 -> 8-pivot Gauss-Jordan on DVE:
    prow = shuffle (u32-bitcast view: half width)
    m    = col_pc(qa) / col_pc(prow)      (tensor_tensor divide)
    mneg = mask_r - m                     (scalar_tensor_tensor)
    qa  += mneg * prow                    (two fp16 2x tensor_tensor)
Forward: batch-on-partition [64, *] fp16 broadcast-mult + reduce per step.
"""

import os
import sys

import numpy as np

for _p in ("/opt/trn_rl_repo",):
    if _p not in sys.path:
        sys.path.insert(0, _p)

import concourse.bass as bass
import concourse.bacc as bacc
import concourse.mybir as mybir
from concourse import tile
from concourse.bass_utils import run_bass_kernel_spmd

F32 = mybir.dt.float32
F16 = mybir.dt.float16
U32 = mybir.dt.uint32
F32R = mybir.dt.float32r
AX = mybir.AxisListType
OP = mybir.AluOpType

B, T, NX, NU = 512, 100, 24, 8
NZ = NX + NU  # 32
NCORES = 8
BC = B // NCORES  # 64 batches per core
G, SL = 4, 16  # partition groups x free slots
W = NZ + 1  # 33 columns (32 matrix + 1 aug)
FW = SL * W  # 528
MATW = SL * NZ  # 512 (matrix part, contiguous cols 0..512)
XOUT = (T + 1) * NX  # 2424
UOUT = T * NU  # 800
OUT_W = XOUT + UOUT  # 3224

LAST_EXEC_NS = None

_prog_cache = {}


def _build_program(t_steps=T):
    nc = bacc.Bacc("TRN2", target_bir_lowering=False, debug=False)
    Ts = t_steps
    XOUT = (Ts + 1) * NX
    UOUT = Ts * NU
    OUT_W = XOUT + UOUT

    # DRAM I/O (fp16 packed host-side)
    caug = nc.dram_tensor("caug", [Ts, 128, FW], F16, kind="ExternalInput")
    vt0 = nc.dram_tensor("vt0", [128, FW], F16, kind="ExternalInput")
    lz16d = nc.dram_tensor("lz16", [128, 128], F16, kind="ExternalInput")
    id16d = nc.dram_tensor("id16", [128, 128], F16, kind="ExternalInput")
    masksd = nc.dram_tensor("masks", [128, NU * SL], F16, kind="ExternalInput")
    x0p = nc.dram_tensor("x0p", [BC, W], F16, kind="ExternalInput")
    abrep = nc.dram_tensor("abrep", [BC, NX * W], F16, kind="ExternalInput")
    out = nc.dram_tensor("out", [BC, OUT_W], F32, kind="ExternalOutput")
    negmd = nc.dram_tensor("negmask", [128, NU], F16, kind="ExternalInput")
    # gains scratch: final u-rows, stored pre-transposed with t innermost
    # per (g,s) so the forward pass loads 4 steps per contiguous DMA
    kbuf = nc.dram_tensor("kbuf", [G, SL, Ts, NU, W], F16)

    with tile.TileContext(nc) as tc:
        with (
            tc.tile_pool(name="const", bufs=1) as cpool,
            tc.tile_pool(name="cstream", bufs=3) as cs_pool,
            tc.tile_pool(name="qa", bufs=2) as qa_pool,
            tc.tile_pool(name="jt", bufs=2) as j_pool,
            tc.tile_pool(name="prow", bufs=4) as pr_pool,
            tc.tile_pool(name="wide", bufs=4) as wide_pool,
            tc.tile_pool(name="small", bufs=8) as sm_pool,
            tc.tile_pool(name="ps_s", bufs=2, space="PSUM") as ps_s,
            tc.tile_pool(name="ps_q", bufs=2, space="PSUM") as ps_q,
            tc.tile_pool(name="ps_v", bufs=2, space="PSUM") as ps_v,
            tc.tile_pool(name="fwd", bufs=1) as f_pool,
            tc.tile_pool(name="kstream", bufs=3) as k_pool,
            tc.tile_pool(name="ftmp", bufs=2) as ft_pool,
        ):
            # ---- constants to SBUF (bounced through DVE for walrus) ----
            def const_tile(name, dram, shape, dt):
                raw = cpool.tile(shape, dt, tag=name + "raw")
                nc.sync.dma_start(out=raw[:], in_=dram[:])
                t_ = cpool.tile(shape, dt, tag=name)
                nc.vector.tensor_copy(out=t_[:], in_=raw[:])
                return t_

            lz16 = const_tile("lz16", lz16d, [128, 128], F16)
            id16 = const_tile("id16", id16d, [128, 128], F16)
            mask_t = cpool.tile([128, NU * SL], F16, tag="masks")
            nc.sync.dma_start(out=mask_t[:], in_=masksd[:])
            negm_t = cpool.tile([128, NU], F16, tag="negmask")
            nc.sync.dma_start(out=negm_t[:], in_=negmd[:])

            # V_T tile
            vcur = const_tile("vterm", vt0, [128, FW], F16)

            def mat_view(t_):  # [128, 512] matrix columns (contiguous)
                return t_[:, 0:MATW]

            def aug_view(t_):  # [128, 16] aug column
                return t_[:, MATW:FW]

            def col_view(t_, c):  # [128, 16] matrix column c
                return t_[:, SL * c : SL * (c + 1)]

            def sc_view(ap):  # free (c,s) -> (s,c) virtual order
                return ap.rearrange("p (c s) -> p s c", s=SL)

            # ---- backward Riccati ----
            for tstep in range(Ts - 1, -1, -1):
                ct = cs_pool.tile([128, FW], F16, tag="ct")
                nc.sync.dma_start(out=ct[:], in_=caug[tstep])

                # S = Z^T V in (s,c) free order (strided rhs) -> PSUM f32
                s_ps = ps_s.tile([128, MATW], F32, tag="s")
                nc.tensor.matmul(
                    out=s_ps[:], lhsT=lz16[:], rhs=sc_view(mat_view(vcur)),
                    start=True, stop=True,
                )
                vq_ps = ps_v.tile([128, SL], F32, tag="vq")
                nc.tensor.matmul(
                    out=vq_ps[:], lhsT=lz16[:], rhs=aug_view(vcur),
                    start=True, stop=False,
                )
                # C lands in PSUM first (off the critical chain: only needs
                # the ct DMA), then Z^T J accumulates on top.
                q_ps = ps_q.tile([128, MATW], F32, tag="q")
                nc.tensor.matmul(
                    out=q_ps[:], lhsT=id16[:], rhs=sc_view(mat_view(ct)),
                    start=True, stop=False,
                )
                nc.tensor.matmul(
                    out=vq_ps[:], lhsT=id16[:], rhs=aug_view(ct),
                    start=False, stop=True,
                )
                # S -> fp16 SBUF via Act (dtype convert), then same-dtype
                # per-block transpose; Th runs as a fast fp16 matmul.
                s16 = j_pool.tile([128, MATW], F16, tag="s16")
                nc.scalar.copy(out=s16[:], in_=s_ps[:])
                j_sb = j_pool.tile([128, MATW], F16, tag="j")
                nc.vector.transpose(out=j_sb[:], in_=s16[:])
                nc.tensor.matmul(
                    out=q_ps[:], lhsT=lz16[:], rhs=j_sb[:],
                    start=False, stop=True,
                )
                # assemble qa (SBUF fp16, (c,s) layout) via Act engine
                qa = qa_pool.tile([128, FW], F16, tag="qa")
                nc.scalar.copy(out=sc_view(mat_view(qa)), in_=q_ps[:])
                nc.scalar.copy(out=aug_view(qa), in_=vq_ps[:])

                # ---- 8-pivot Gauss-Jordan on the u block ----
                for r in range(NU):
                    pc = NX + r
                    prow = pr_pool.tile([128, FW], F16, tag="prow")
                    nc.vector.stream_shuffle(
                        out=prow[:].bitcast(U32),
                        in_=qa[:].bitcast(U32),
                        mask=[pc] * 32,
                    )
                    # mneg = (col_pc(qa) * negmask_r) * (1/piv):
                    # negmask_r is -1 per partition, 0 on pivot rows, so the
                    # pivot row is preserved (mneg=0) without a mask add.
                    drec = sm_pool.tile([128, SL], F16, tag="drec")
                    with nc.allow_low_precision(reason="piv >= 1"):
                        nc.vector.reciprocal(
                            out=drec[:], in_=col_view(prow, pc)
                        )
                    mneg = sm_pool.tile([128, SL], F16, tag="mneg")
                    nc.vector.scalar_tensor_tensor(
                        out=mneg[:], in0=col_view(qa, pc),
                        scalar=negm_t[:, r : r + 1],
                        in1=drec[:],
                        op0=OP.mult, op1=OP.mult,
                    )
                    tmp = wide_pool.tile([128, FW], F16, tag="tmp")
                    mneg_b = mneg[:].unsqueeze(1).broadcast_to((128, W, SL))
                    nc.vector.tensor_tensor(
                        out=tmp[:].rearrange("p (c s) -> p c s", s=SL),
                        in0=mneg_b,
                        in1=prow[:].rearrange("p (c s) -> p c s", s=SL),
                        op=OP.mult,
                    )
                    nc.vector.tensor_tensor(
                        out=qa[:], in0=qa[:], in1=tmp[:], op=OP.add
                    )

                # store gain rows (final u-rows), transposed to [s][t][r,c]
                # so the forward read is one contiguous block per (g,s)
                for g in range(G):
                    kb0 = kbuf[g][:]
                    dst = bass.AP(
                        tensor=kb0.tensor,
                        offset=kb0.offset + tstep * (NU * W),
                        ap=[
                            [W, NU], [1, W],       # (r, c)
                            [Ts * NU * W, SL],     # s
                        ],
                    )
                    nc.sync.dma_start(
                        out=dst,
                        in_=qa[32 * g + NX : 32 * g + NZ, :].rearrange(
                            "r (c s) -> r c s", s=SL
                        ),
                    )
                vcur = qa

            # ---- forward rollout (batch on partitions, fp16) ----
            xt = f_pool.tile([BC, W], F16, tag="xt")  # [x | u(=0) | 1]
            nc.sync.dma_start(out=xt[:], in_=x0p[:])
            ab_t = cpool.tile([BC, NX * W], F16, tag="abrep")
            nc.sync.dma_start(out=ab_t[:], in_=abrep[:])
            xall = f_pool.tile([BC, XOUT], F32, tag="xall")
            uall = f_pool.tile([BC, UOUT], F32, tag="uall")

            KQ = 4  # forward steps per kt DMA
            assert Ts % KQ == 0 or Ts < KQ
            kq_eff = KQ if Ts >= KQ else Ts
            kt4 = None
            for tstep in range(Ts):
                j = tstep % kq_eff
                if j == 0:
                    # one contiguous load of kq_eff steps of gains:
                    # kt4[(g,s), (t', r, c)]
                    kt4 = k_pool.tile([BC, kq_eff * NU * W], F16, tag="kt4")
                    kb0 = kbuf[0][:]
                    src = bass.AP(
                        tensor=kb0.tensor,
                        offset=kb0.offset + tstep * (NU * W),
                        ap=[
                            [SL * Ts * NU * W, G], [Ts * NU * W, SL],
                            [1, kq_eff * NU * W],
                        ],
                    )
                    nc.sync.dma_start(out=kt4[:], in_=src)
                kt = kt4[:, j * (NU * W) : (j + 1) * (NU * W)]
                # record x_t (f32 out)
                nc.scalar.copy(
                    out=xall[:, tstep * NX : (tstep + 1) * NX], in_=xt[:, 0:NX]
                )
                # s_r = sum_c kt[r, c] * z[c]   (u slots of z are zero)
                t0 = ft_pool.tile([BC, NU * W], F16, tag="t0")
                nc.vector.tensor_tensor(
                    out=t0[:].rearrange("p (r c) -> p r c", c=W),
                    in0=kt.rearrange("p (r c) -> p r c", c=W),
                    in1=xt[:].unsqueeze(1).broadcast_to((BC, NU, W)),
                    op=OP.mult,
                )
                ssum = ft_pool.tile([BC, NU], F32, tag="ssum")
                nc.vector.tensor_reduce(
                    out=ssum[:],
                    in_=t0[:].rearrange("p (r c) -> p r c", c=W),
                    axis=AX.X, op=OP.add,
                )
                # u = -s / pivot ; pivot at kt[r, 24+r] -> flat 33r + 24 + r
                pivd = bass.AP(
                    tensor=kt.tensor,
                    offset=kt.offset + NX,
                    ap=[list(kt.ap[0]), [W + 1, NU]],
                )
                prec = ft_pool.tile([BC, NU], F32, tag="prec")
                nc.vector.reciprocal(out=prec[:], in_=pivd)
                ut = ft_pool.tile([BC, NU], F16, tag="ut")
                nc.vector.scalar_tensor_tensor(
                    out=ut[:], in0=ssum[:], scalar=-1.0, in1=prec[:],
                    op0=OP.mult, op1=OP.mult,
                )
                nc.scalar.copy(
                    out=uall[:, tstep * NU : (tstep + 1) * NU], in_=ut[:]
                )
                # z = [x | u | 1]
                nc.vector.tensor_copy(out=xt[:, NX:NZ], in_=ut[:])
                # x' = [A B 0] z
                t2 = ft_pool.tile([BC, NX * W], F16, tag="t2")
                nc.vector.tensor_tensor(
                    out=t2[:].rearrange("p (i c) -> p i c", c=W),
                    in0=ab_t[:].rearrange("p (i c) -> p i c", c=W),
                    in1=xt[:].unsqueeze(1).broadcast_to((BC, NX, W)),
                    op=OP.mult,
                )
                with nc.allow_low_precision(reason="fp16 state"):
                    nc.vector.tensor_reduce(
                        out=xt[:, 0:NX],
                        in_=t2[:].rearrange("p (i c) -> p i c", c=W),
                        axis=AX.X, op=OP.add,
                    )
                # restore z invariant [x | 0 | 1] for the next step's s_r
                nc.vector.memset(xt[:, NX:NZ], 0.0)
            nc.scalar.copy(out=xall[:, Ts * NX : (Ts + 1) * NX], in_=xt[:, 0:NX])
            nc.sync.dma_start(out=out[:, 0:XOUT], in_=xall[:])
            nc.sync.dma_start(out=out[:, XOUT:OUT_W], in_=uall[:])

    nc.compile()
    return nc


def _host_pack(inputs, t_steps=T):
    """q precompute + per-core col-major GRM packing (fp16). Returns in_maps."""
    Ts = t_steps
    x0 = np.asarray(inputs["x0"], np.float32)
    C = np.asarray(inputs["C"], np.float32)[:, :Ts]
    c = np.asarray(inputs["c"], np.float32)[:, :Ts]
    C_final = np.asarray(inputs["C_final"], np.float32)
    c_final = np.asarray(inputs["c_final"], np.float32)
    x_ref = np.asarray(inputs["x_ref"], np.float32)
    u_ref = np.asarray(inputs["u_ref"], np.float32)[:, :Ts]
    A = np.asarray(inputs["A_dyn"], np.float32)
    Bd = np.asarray(inputs["B_dyn"], np.float32)

    xr = x_ref[:, : Ts + 1] if x_ref.shape[1] > Ts else x_ref
    zref = np.concatenate([xr[:, :Ts], u_ref], axis=-1)  # [B,Ts,32]
    q = c - np.einsum("btij,btj->bti", C, zref)
    VT = C_final[:, :NX, :NX]
    vT = c_final[:, :NX] - np.einsum("bij,bj->bi", VT, xr[:, Ts])

    # caug [cores, Ts, G, 32(i), 33(c), SL(s)]
    caug = np.zeros((NCORES, Ts, G, NZ, W, SL), np.float32)
    Cb = C.reshape(NCORES, G, SL, Ts, NZ, NZ)
    caug[..., 0:NZ, :] = Cb.transpose(0, 3, 1, 4, 5, 2)
    qb = q.reshape(NCORES, G, SL, Ts, NZ)
    caug[..., NZ, :] = qb.transpose(0, 3, 1, 4, 2)
    caug = np.ascontiguousarray(
        caug.reshape(NCORES, Ts, 128, FW)
    ).astype(np.float16)

    vt0 = np.zeros((NCORES, G, NZ, W, SL), np.float32)
    VTb = VT.reshape(NCORES, G, SL, NX, NX)
    vt0[:, :, 0:NX, 0:NX, :] = VTb.transpose(0, 1, 3, 4, 2)
    vTb = vT.reshape(NCORES, G, SL, NX)
    vt0[:, :, 0:NX, NZ, :] = vTb.transpose(0, 1, 3, 2)
    vt0 = np.ascontiguousarray(vt0.reshape(NCORES, 128, FW)).astype(np.float16)

    AB = np.concatenate([A, Bd], axis=1)  # [24, 32]
    Zpad = np.zeros((NZ, NZ), np.float32)
    Zpad[0:NX, :] = AB
    lz = np.zeros((128, 128), np.float32)
    for g in range(G):
        lz[32 * g : 32 * g + NZ, 32 * g : 32 * g + NZ] = Zpad
    lz16 = lz.astype(np.float16)
    id16 = np.eye(128, dtype=np.float16)

    masks = np.zeros((128, NU * SL), np.float16)
    for r in range(NU):
        for g in range(G):
            masks[32 * g + NX + r, r * SL : (r + 1) * SL] = 1.0
    negmask = np.full((128, NU), -1.0, np.float16)
    for r in range(NU):
        for g in range(G):
            negmask[32 * g + NX + r, r] = 0.0

    x0p = np.zeros((NCORES, BC, W), np.float32)
    x0b = x0.reshape(NCORES, G, SL, NX)
    for g in range(G):
        x0p[:, g * SL : (g + 1) * SL, 0:NX] = x0b[:, g]
    x0p[:, :, NZ] = 1.0
    x0p = x0p.astype(np.float16)

    abaug = np.zeros((NX, W), np.float32)
    abaug[:, 0:NZ] = AB
    abrep = np.ascontiguousarray(
        np.broadcast_to(abaug.reshape(1, NX * W), (BC, NX * W))
    ).astype(np.float16)

    in_maps = []
    for core in range(NCORES):
        in_maps.append(
            {
                "caug": caug[core],
                "vt0": vt0[core],
                "lz16": lz16,
                "id16": id16,
                "masks": masks,
                "negmask": negmask,
                "x0p": x0p[core],
                "abrep": abrep[core] if abrep.ndim == 3 else abrep,
            }
        )
    return in_maps


def _unpack(results):
    outs = [results[core]["out"] for core in range(NCORES)]
    return np.concatenate(outs, axis=0)


def kernel(**inputs):
    global LAST_EXEC_NS
    trace = bool(int(os.environ.get("KERNEL_TRACE", "0")))
    key = ("prog", trace)
    if key not in _prog_cache:
        _prog_cache[key] = _build_program()
    nc = _prog_cache[key]
    in_maps = _host_pack(inputs)
    res = run_bass_kernel_spmd(
        nc, in_maps, core_ids=list(range(NCORES)), trace=trace
    )
    LAST_EXEC_NS = res.exec_time_ns
    return _unpack(res.results)


def bench(inputs, iters=10):
    """Device-resident repeated execution timing (same path as baseline)."""
    import time

    import jax
    from jax.sharding import Mesh, NamedSharding, PartitionSpec
    from jax.experimental.shard_map import shard_map

    from concourse import bass2jax as B2J

    key = ("prog", False)
    if key not in _prog_cache:
        _prog_cache[key] = _build_program()
    nc = _prog_cache[key]
    in_maps = _host_pack(inputs)

    B2J.install_neuronx_cc_hook()
    in_names, out_names, out_avals, zero_outs = [], [], [], []
    for alloc in nc.m.functions[0].allocations:
        if not isinstance(alloc, mybir.MemoryLocationSet):
            continue
        name = alloc.memorylocations[0].name
        if alloc.kind == "ExternalInput":
            if (
                nc.partition_id_tensor is not None
                and name == nc.partition_id_tensor.name
            ):
                continue
            in_names.append(name)
        elif alloc.kind == "ExternalOutput":
            out_names.append(name)
            shape = tuple(alloc.tensor_shape)
            dtype = mybir.dt.np(alloc.dtype)
            out_avals.append(jax.core.ShapedArray(shape, dtype))
            zero_outs.append(np.zeros(shape, dtype))
    n_params = len(in_names)
    all_in_names = list(in_names) + list(out_names)
    partition_name = (
        nc.partition_id_tensor.name if nc.partition_id_tensor else None
    )
    if partition_name is not None:
        all_in_names.append(partition_name)

    def _body(*args):
        operands = list(args)
        if partition_name is not None:
            operands.append(B2J.partition_id_tensor())
        outs = B2J._bass_exec_p.bind(
            *operands,
            out_avals=tuple(out_avals),
            in_names=tuple(all_in_names),
            out_names=tuple(out_names),
            lowering_input_output_aliases=(),
            sim_require_finite=True,
            sim_require_nnan=True,
            nc=nc,
        )
        return tuple(outs)

    devices = jax.devices()[:NCORES]
    mesh = Mesh(np.asarray(devices), ("core",))
    nops = n_params + len(out_names)
    sharded = jax.jit(
        shard_map(
            _body,
            mesh=mesh,
            in_specs=(PartitionSpec("core"),) * nops,
            out_specs=(PartitionSpec("core"),) * len(out_names),
            check_rep=False,
        ),
        keep_unused=True,
    )
    sh = NamedSharding(mesh, PartitionSpec("core"))
    dev_in = [
        jax.device_put(
            np.concatenate(
                [np.asarray(in_maps[c][n]) for c in range(NCORES)], axis=0
            ),
            sh,
        )
        for n in in_names
    ]
    dev_zero = [
        jax.device_put(
            np.zeros((NCORES * z.shape[0], *z.shape[1:]), z.dtype), sh
        )
        for z in zero_outs
    ]
    outs = sharded(*dev_in, *dev_zero)
    jax.block_until_ready(outs)
    best = float("inf")
    for _ in range(iters):
        t0 = time.perf_counter()
        outs = sharded(*dev_in, *dev_zero)
        jax.block_until_ready(outs)
        best = min(best, time.perf_counter() - t0)
    full = np.asarray(outs[out_names.index("out")])
    return best, full


# revision 6
# speedup vs baseline: 1.1162x; 1.1162x over previous
"""Trainium2 Bass kernel v2 for batched differentiable-MPC (LQR) controller.

Riccati backward sweep + forward rollout, B=512 data-parallel over 8 cores
(64 batches/core).

v2 layout ("col-major GRM"): per core, local batch b = 16*g + s with
partition-group g in [0,4) and slot s in [0,16). A per-batch 32x32 z-space
matrix M_b and augmented column q_b live in a [128, 528] fp16 tile:
    tile[32*g + i, 16*c + s] = M_b[i, c]   (c < 32)
    tile[32*g + i, 512 + s]  = q_b[i]      (aug col c=32)
Col-major slots make every column view contiguous and keep the DVE 2x fp16
mode on the broadcast FMA (broadcast over the middle dim, packed last dim).

Backward step: S = Z^T V (PE, fp16) -> J = per-block transpose (DVE, f32
PSUM->SBUF) -> Q = C + Z^T J, qz = q + Z^T v (PE accumulate) -> Act copies
PSUM->SBUF fp16 -> 8-pivot Gauss-Jordan on DVE:
    prow = shuffle (u32-bitcast view: half width)
    m    = col_pc(qa) / col_pc(prow)      (tensor_tensor divide)
    mneg = mask_r - m                     (scalar_tensor_tensor)
    qa  += mneg * prow                    (two fp16 2x tensor_tensor)
Forward: batch-on-partition [64, *] fp16 broadcast-mult + reduce per step.
"""

import os
import sys

import numpy as np

for _p in ("/opt/trn_rl_repo",):
    if _p not in sys.path:
        sys.path.insert(0, _p)

import concourse.bass as bass
import concourse.bacc as bacc
import concourse.mybir as mybir
from concourse import tile
from concourse.bass_utils import run_bass_kernel_spmd

F32 = mybir.dt.float32
F16 = mybir.dt.float16
U32 = mybir.dt.uint32
F32R = mybir.dt.float32r
AX = mybir.AxisListType
OP = mybir.AluOpType

B, T, NX, NU = 512, 100, 24, 8
NZ = NX + NU  # 32
NCORES = 8
BC = B // NCORES  # 64 batches per core
G, SL = 4, 16  # partition groups x free slots
W = NZ + 1  # 33 columns (32 matrix + 1 aug)
FW = SL * W  # 528
MATW = SL * NZ  # 512 (matrix part, contiguous cols 0..512)
XOUT = (T + 1) * NX  # 2424
UOUT = T * NU  # 800
OUT_W = XOUT + UOUT  # 3224

LAST_EXEC_NS = None

_prog_cache = {}


def _build_program(t_steps=T):
    nc = bacc.Bacc("TRN2", target_bir_lowering=False, debug=False)
    Ts = t_steps
    XOUT = (Ts + 1) * NX
    UOUT = Ts * NU
    OUT_W = XOUT + UOUT

    # DRAM I/O (fp16 packed host-side)
    caug = nc.dram_tensor("caug", [Ts, 128, FW], F16, kind="ExternalInput")
    vt0 = nc.dram_tensor("vt0", [128, FW], F16, kind="ExternalInput")
    lz16d = nc.dram_tensor("lz16", [128, 128], F16, kind="ExternalInput")
    id16d = nc.dram_tensor("id16", [128, 128], F16, kind="ExternalInput")
    masksd = nc.dram_tensor("masks", [128, NU * SL], F16, kind="ExternalInput")
    x0p = nc.dram_tensor("x0p", [BC, W], F16, kind="ExternalInput")
    abrep = nc.dram_tensor("abrep", [BC, NX * W], F16, kind="ExternalInput")
    out = nc.dram_tensor("out", [BC, OUT_W], F32, kind="ExternalOutput")
    negmd = nc.dram_tensor("negmask", [128, NU], F16, kind="ExternalInput")
    # gains scratch: final u-rows repacked to (s,c) rows on-chip, so both
    # the store and the forward gather use >=66B-contiguous DMA runs
    kbuf = nc.dram_tensor("kbuf", [G, Ts, NU, W * SL], F16)

    with tile.TileContext(nc) as tc:
        with (
            tc.tile_pool(name="const", bufs=1) as cpool,
            tc.tile_pool(name="cstream", bufs=3) as cs_pool,
            tc.tile_pool(name="qa", bufs=2) as qa_pool,
            tc.tile_pool(name="jt", bufs=2) as j_pool,
            tc.tile_pool(name="prow", bufs=4) as pr_pool,
            tc.tile_pool(name="wide", bufs=4) as wide_pool,
            tc.tile_pool(name="small", bufs=8) as sm_pool,
            tc.tile_pool(name="ps_s", bufs=2, space="PSUM") as ps_s,
            tc.tile_pool(name="ps_q0", bufs=2, space="PSUM") as ps_q0,
            tc.tile_pool(name="ps_q1", bufs=2, space="PSUM") as ps_q1,
            tc.tile_pool(name="ps_v", bufs=2, space="PSUM") as ps_v,
            tc.tile_pool(name="fwd", bufs=1) as f_pool,
            tc.tile_pool(name="kstream", bufs=3) as k_pool,
            tc.tile_pool(name="ftmp", bufs=2) as ft_pool,
        ):
            # ---- constants to SBUF (bounced through DVE for walrus) ----
            def const_tile(name, dram, shape, dt):
                raw = cpool.tile(shape, dt, tag=name + "raw")
                nc.sync.dma_start(out=raw[:], in_=dram[:])
                t_ = cpool.tile(shape, dt, tag=name)
                nc.vector.tensor_copy(out=t_[:], in_=raw[:])
                return t_

            lz16 = const_tile("lz16", lz16d, [128, 128], F16)
            id16 = const_tile("id16", id16d, [128, 128], F16)
            mask_t = cpool.tile([128, NU * SL], F16, tag="masks")
            nc.sync.dma_start(out=mask_t[:], in_=masksd[:])
            negm_t = cpool.tile([128, NU], F16, tag="negmask")
            nc.sync.dma_start(out=negm_t[:], in_=negmd[:])

            # V_T tile
            vcur = const_tile("vterm", vt0, [128, FW], F16)

            def mat_view(t_):  # [128, 512] matrix columns (contiguous)
                return t_[:, 0:MATW]

            def aug_view(t_):  # [128, 16] aug column
                return t_[:, MATW:FW]

            def col_view(t_, c):  # [128, 16] matrix column c
                return t_[:, SL * c : SL * (c + 1)]

            def sc_view(ap):  # free (c,s) -> (s,c) virtual order
                return ap.rearrange("p (c s) -> p s c", s=SL)

            # ---- backward Riccati ----
            for tstep in range(Ts - 1, -1, -1):
                ct = cs_pool.tile([128, FW], F16, tag="ct")
                nc.sync.dma_start(out=ct[:], in_=caug[tstep])

                # The (s,c)-ordered mat pipeline is split into two slot
                # halves (s 0..7 / 8..15 = flat col blocks 0:256 / 256:512)
                # so S -> s16 -> transpose -> Th -> qa-copy pipelines across
                # PE / Act / DVE instead of serializing at full width.
                HH = MATW // 2  # 256

                vq_ps = ps_v.tile([128, SL], F32, tag="vq")
                nc.tensor.matmul(
                    out=vq_ps[:], lhsT=lz16[:], rhs=aug_view(vcur),
                    start=True, stop=False,
                )
                # C lands in PSUM first (off the critical chain: only needs
                # the ct DMA), then Z^T J accumulates on top. Each half is
                # its own accumulation group (own PSUM bank) so its qa copy
                # can start as soon as that half's Th lands.
                ct_sc = sc_view(mat_view(ct))
                q_h = []
                for h, pool in ((0, ps_q0), (1, ps_q1)):
                    qp = pool.tile([128, HH], F32, tag=f"q{h}")
                    q_h.append(qp)
                    nc.tensor.matmul(
                        out=qp[:], lhsT=id16[:],
                        rhs=ct_sc[:, h * 8 : (h + 1) * 8, :],
                        start=True, stop=False,
                    )
                nc.tensor.matmul(
                    out=vq_ps[:], lhsT=id16[:], rhs=aug_view(ct),
                    start=False, stop=True,
                )
                s_ps = ps_s.tile([128, MATW], F32, tag="s")
                s16 = j_pool.tile([128, MATW], F16, tag="s16")
                j_sb = j_pool.tile([128, MATW], F16, tag="j")
                qa = qa_pool.tile([128, FW], F16, tag="qa")
                rhs_sc = sc_view(mat_view(vcur))
                for h in (0, 1):
                    hs = slice(h * HH, (h + 1) * HH)
                    nc.tensor.matmul(
                        out=s_ps[:, hs], lhsT=lz16[:],
                        rhs=rhs_sc[:, h * 8 : (h + 1) * 8, :],
                        start=True, stop=True,
                    )
                    nc.scalar.copy(out=s16[:, hs], in_=s_ps[:, hs])
                    nc.vector.transpose(out=j_sb[:, hs], in_=s16[:, hs])
                    nc.tensor.matmul(
                        out=q_h[h][:], lhsT=lz16[:], rhs=j_sb[:, hs],
                        start=False, stop=True,
                    )
                    # qa mat half: strided (c,s) target, slots h*8..h*8+8
                    qa_half = mat_view(qa).rearrange(
                        "p (c s) -> p s c", s=SL
                    )[:, h * 8 : (h + 1) * 8, :]
                    nc.scalar.copy(out=qa_half, in_=q_h[h][:])
                nc.scalar.copy(out=aug_view(qa), in_=vq_ps[:])

                # ---- 8-pivot Gauss-Jordan on the u block ----
                for r in range(NU):
                    pc = NX + r
                    prow = pr_pool.tile([128, FW], F16, tag="prow")
                    nc.vector.stream_shuffle(
                        out=prow[:].bitcast(U32),
                        in_=qa[:].bitcast(U32),
                        mask=[pc] * 32,
                    )
                    # mneg = (col_pc(qa) * negmask_r) * (1/piv):
                    # negmask_r is -1 per partition, 0 on pivot rows, so the
                    # pivot row is preserved (mneg=0) without a mask add.
                    drec = sm_pool.tile([128, SL], F16, tag="drec")
                    with nc.allow_low_precision(reason="piv >= 1"):
                        nc.vector.reciprocal(
                            out=drec[:], in_=col_view(prow, pc)
                        )
                    mneg = sm_pool.tile([128, SL], F16, tag="mneg")
                    nc.vector.scalar_tensor_tensor(
                        out=mneg[:], in0=col_view(qa, pc),
                        scalar=negm_t[:, r : r + 1],
                        in1=drec[:],
                        op0=OP.mult, op1=OP.mult,
                    )
                    tmp = wide_pool.tile([128, FW], F16, tag="tmp")
                    mneg_b = mneg[:].unsqueeze(1).broadcast_to((128, W, SL))
                    nc.vector.tensor_tensor(
                        out=tmp[:].rearrange("p (c s) -> p c s", s=SL),
                        in0=mneg_b,
                        in1=prow[:].rearrange("p (c s) -> p c s", s=SL),
                        op=OP.mult,
                    )
                    nc.vector.tensor_tensor(
                        out=qa[:], in0=qa[:], in1=tmp[:], op=OP.add
                    )

                # repack gain rows to (s,c) order on the idle Pool engine
                # (off the critical path), then store contiguous rows
                kt_rep = pr_pool.tile([128, FW], F16, tag="ktrep")
                nc.gpsimd.tensor_copy(
                    out=kt_rep[:],
                    in_=qa[:].rearrange("p (c s) -> p s c", s=SL),
                )
                for g in range(G):
                    nc.sync.dma_start(
                        out=kbuf[g, tstep],
                        in_=kt_rep[32 * g + NX : 32 * g + NZ, :],
                    )
                vcur = qa

            # ---- forward rollout (batch on partitions, fp16) ----
            xt = f_pool.tile([BC, W], F16, tag="xt")  # [x | u(=0) | 1]
            nc.sync.dma_start(out=xt[:], in_=x0p[:])
            ab_t = cpool.tile([BC, NX * W], F16, tag="abrep")
            nc.sync.dma_start(out=ab_t[:], in_=abrep[:])
            xall = f_pool.tile([BC, XOUT], F32, tag="xall")
            uall = f_pool.tile([BC, UOUT], F32, tag="uall")

            KQ = 4  # forward steps per kt DMA
            assert Ts % KQ == 0 or Ts < KQ
            kq_eff = KQ if Ts >= KQ else Ts
            kt4 = None
            for tstep in range(Ts):
                j = tstep % kq_eff
                if j == 0:
                    # per group: kq_eff steps of gains in one DMA with
                    # 33-element contiguous runs: kt4[(g,s), ((t',r), c)]
                    kt4 = k_pool.tile([BC, kq_eff * NU * W], F16, tag="kt4")
                    for g in range(G):
                        kb0 = kbuf[g][:]
                        src = bass.AP(
                            tensor=kb0.tensor,
                            offset=kb0.offset + tstep * (NU * W * SL),
                            ap=[
                                [W, SL],                    # s -> partition
                                [W * SL, kq_eff * NU],      # (t', r)
                                [1, W],                     # c contiguous
                            ],
                        )
                        nc.sync.dma_start(
                            out=kt4[g * SL : (g + 1) * SL, :], in_=src
                        )
                kt = kt4[:, j * (NU * W) : (j + 1) * (NU * W)]
                # record x_t (f32 out)
                nc.scalar.copy(
                    out=xall[:, tstep * NX : (tstep + 1) * NX], in_=xt[:, 0:NX]
                )
                # s_r = sum_c kt[r, c] * z[c]   (u slots of z are zero)
                t0 = ft_pool.tile([BC, NU * W], F16, tag="t0")
                nc.vector.tensor_tensor(
                    out=t0[:].rearrange("p (r c) -> p r c", c=W),
                    in0=kt.rearrange("p (r c) -> p r c", c=W),
                    in1=xt[:].unsqueeze(1).broadcast_to((BC, NU, W)),
                    op=OP.mult,
                )
                ssum = ft_pool.tile([BC, NU], F32, tag="ssum")
                nc.vector.tensor_reduce(
                    out=ssum[:],
                    in_=t0[:].rearrange("p (r c) -> p r c", c=W),
                    axis=AX.X, op=OP.add,
                )
                # u = -s / pivot ; pivot at kt[r, 24+r] -> flat 33r + 24 + r
                pivd = bass.AP(
                    tensor=kt.tensor,
                    offset=kt.offset + NX,
                    ap=[list(kt.ap[0]), [W + 1, NU]],
                )
                prec = ft_pool.tile([BC, NU], F32, tag="prec")
                nc.vector.reciprocal(out=prec[:], in_=pivd)
                ut = ft_pool.tile([BC, NU], F16, tag="ut")
                nc.vector.scalar_tensor_tensor(
                    out=ut[:], in0=ssum[:], scalar=-1.0, in1=prec[:],
                    op0=OP.mult, op1=OP.mult,
                )
                nc.scalar.copy(
                    out=uall[:, tstep * NU : (tstep + 1) * NU], in_=ut[:]
                )
                # z = [x | u | 1]
                nc.vector.tensor_copy(out=xt[:, NX:NZ], in_=ut[:])
                # x' = [A B 0] z
                t2 = ft_pool.tile([BC, NX * W], F16, tag="t2")
                nc.vector.tensor_tensor(
                    out=t2[:].rearrange("p (i c) -> p i c", c=W),
                    in0=ab_t[:].rearrange("p (i c) -> p i c", c=W),
                    in1=xt[:].unsqueeze(1).broadcast_to((BC, NX, W)),
                    op=OP.mult,
                )
                with nc.allow_low_precision(reason="fp16 state"):
                    nc.vector.tensor_reduce(
                        out=xt[:, 0:NX],
                        in_=t2[:].rearrange("p (i c) -> p i c", c=W),
                        axis=AX.X, op=OP.add,
                    )
                # restore z invariant [x | 0 | 1] for the next step's s_r
                nc.vector.memset(xt[:, NX:NZ], 0.0)
            nc.scalar.copy(out=xall[:, Ts * NX : (Ts + 1) * NX], in_=xt[:, 0:NX])
            nc.sync.dma_start(out=out[:, 0:XOUT], in_=xall[:])
            nc.sync.dma_start(out=out[:, XOUT:OUT_W], in_=uall[:])

    nc.compile()
    return nc


def _host_pack(inputs, t_steps=T):
    """q precompute + per-core col-major GRM packing (fp16). Returns in_maps."""
    Ts = t_steps
    x0 = np.asarray(inputs["x0"], np.float32)
    C = np.asarray(inputs["C"], np.float32)[:, :Ts]
    c = np.asarray(inputs["c"], np.float32)[:, :Ts]
    C_final = np.asarray(inputs["C_final"], np.float32)
    c_final = np.asarray(inputs["c_final"], np.float32)
    x_ref = np.asarray(inputs["x_ref"], np.float32)
    u_ref = np.asarray(inputs["u_ref"], np.float32)[:, :Ts]
    A = np.asarray(inputs["A_dyn"], np.float32)
    Bd = np.asarray(inputs["B_dyn"], np.float32)

    xr = x_ref[:, : Ts + 1] if x_ref.shape[1] > Ts else x_ref
    zref = np.concatenate([xr[:, :Ts], u_ref], axis=-1)  # [B,Ts,32]
    q = c - np.einsum("btij,btj->bti", C, zref)
    VT = C_final[:, :NX, :NX]
    vT = c_final[:, :NX] - np.einsum("bij,bj->bi", VT, xr[:, Ts])

    # caug [cores, Ts, G, 32(i), 33(c), SL(s)]
    caug = np.zeros((NCORES, Ts, G, NZ, W, SL), np.float32)
    Cb = C.reshape(NCORES, G, SL, Ts, NZ, NZ)
    caug[..., 0:NZ, :] = Cb.transpose(0, 3, 1, 4, 5, 2)
    qb = q.reshape(NCORES, G, SL, Ts, NZ)
    caug[..., NZ, :] = qb.transpose(0, 3, 1, 4, 2)
    caug = np.ascontiguousarray(
        caug.reshape(NCORES, Ts, 128, FW)
    ).astype(np.float16)

    vt0 = np.zeros((NCORES, G, NZ, W, SL), np.float32)
    VTb = VT.reshape(NCORES, G, SL, NX, NX)
    vt0[:, :, 0:NX, 0:NX, :] = VTb.transpose(0, 1, 3, 4, 2)
    vTb = vT.reshape(NCORES, G, SL, NX)
    vt0[:, :, 0:NX, NZ, :] = vTb.transpose(0, 1, 3, 2)
    vt0 = np.ascontiguousarray(vt0.reshape(NCORES, 128, FW)).astype(np.float16)

    AB = np.concatenate([A, Bd], axis=1)  # [24, 32]
    Zpad = np.zeros((NZ, NZ), np.float32)
    Zpad[0:NX, :] = AB
    lz = np.zeros((128, 128), np.float32)
    for g in range(G):
        lz[32 * g : 32 * g + NZ, 32 * g : 32 * g + NZ] = Zpad
    lz16 = lz.astype(np.float16)
    id16 = np.eye(128, dtype=np.float16)

    masks = np.zeros((128, NU * SL), np.float16)
    for r in range(NU):
        for g in range(G):
            masks[32 * g + NX + r, r * SL : (r + 1) * SL] = 1.0
    negmask = np.full((128, NU), -1.0, np.float16)
    for r in range(NU):
        for g in range(G):
            negmask[32 * g + NX + r, r] = 0.0

    x0p = np.zeros((NCORES, BC, W), np.float32)
    x0b = x0.reshape(NCORES, G, SL, NX)
    for g in range(G):
        x0p[:, g * SL : (g + 1) * SL, 0:NX] = x0b[:, g]
    x0p[:, :, NZ] = 1.0
    x0p = x0p.astype(np.float16)

    abaug = np.zeros((NX, W), np.float32)
    abaug[:, 0:NZ] = AB
    abrep = np.ascontiguousarray(
        np.broadcast_to(abaug.reshape(1, NX * W), (BC, NX * W))
    ).astype(np.float16)

    in_maps = []
    for core in range(NCORES):
        in_maps.append(
            {
                "caug": caug[core],
                "vt0": vt0[core],
                "lz16": lz16,
                "id16": id16,
                "masks": masks,
                "negmask": negmask,
                "x0p": x0p[core],
                "abrep": abrep[core] if abrep.ndim == 3 else abrep,
            }
        )
    return in_maps


def _unpack(results):
    outs = [results[core]["out"] for core in range(NCORES)]
    return np.concatenate(outs, axis=0)


def kernel(**inputs):
    global LAST_EXEC_NS
    trace = bool(int(os.environ.get("KERNEL_TRACE", "0")))
    key = ("prog", trace)
    if key not in _prog_cache:
        _prog_cache[key] = _build_program()
    nc = _prog_cache[key]
    in_maps = _host_pack(inputs)
    res = run_bass_kernel_spmd(
        nc, in_maps, core_ids=list(range(NCORES)), trace=trace
    )
    LAST_EXEC_NS = res.exec_time_ns
    return _unpack(res.results)


def bench(inputs, iters=10):
    """Device-resident repeated execution timing (same path as baseline)."""
    import time

    import jax
    from jax.sharding import Mesh, NamedSharding, PartitionSpec
    from jax.experimental.shard_map import shard_map

    from concourse import bass2jax as B2J

    key = ("prog", False)
    if key not in _prog_cache:
        _prog_cache[key] = _build_program()
    nc = _prog_cache[key]
    in_maps = _host_pack(inputs)

    B2J.install_neuronx_cc_hook()
    in_names, out_names, out_avals, zero_outs = [], [], [], []
    for alloc in nc.m.functions[0].allocations:
        if not isinstance(alloc, mybir.MemoryLocationSet):
            continue
        name = alloc.memorylocations[0].name
        if alloc.kind == "ExternalInput":
            if (
                nc.partition_id_tensor is not None
                and name == nc.partition_id_tensor.name
            ):
                continue
            in_names.append(name)
        elif alloc.kind == "ExternalOutput":
            out_names.append(name)
            shape = tuple(alloc.tensor_shape)
            dtype = mybir.dt.np(alloc.dtype)
            out_avals.append(jax.core.ShapedArray(shape, dtype))
            zero_outs.append(np.zeros(shape, dtype))
    n_params = len(in_names)
    all_in_names = list(in_names) + list(out_names)
    partition_name = (
        nc.partition_id_tensor.name if nc.partition_id_tensor else None
    )
    if partition_name is not None:
        all_in_names.append(partition_name)

    def _body(*args):
        operands = list(args)
        if partition_name is not None:
            operands.append(B2J.partition_id_tensor())
        outs = B2J._bass_exec_p.bind(
            *operands,
            out_avals=tuple(out_avals),
            in_names=tuple(all_in_names),
            out_names=tuple(out_names),
            lowering_input_output_aliases=(),
            sim_require_finite=True,
            sim_require_nnan=True,
            nc=nc,
        )
        return tuple(outs)

    devices = jax.devices()[:NCORES]
    mesh = Mesh(np.asarray(devices), ("core",))
    nops = n_params + len(out_names)
    sharded = jax.jit(
        shard_map(
            _body,
            mesh=mesh,
            in_specs=(PartitionSpec("core"),) * nops,
            out_specs=(PartitionSpec("core"),) * len(out_names),
            check_rep=False,
        ),
        keep_unused=True,
    )
    sh = NamedSharding(mesh, PartitionSpec("core"))
    dev_in = [
        jax.device_put(
            np.concatenate(
                [np.asarray(in_maps[c][n]) for c in range(NCORES)], axis=0
            ),
            sh,
        )
        for n in in_names
    ]
    dev_zero = [
        jax.device_put(
            np.zeros((NCORES * z.shape[0], *z.shape[1:]), z.dtype), sh
        )
        for z in zero_outs
    ]
    outs = sharded(*dev_in, *dev_zero)
    jax.block_until_ready(outs)
    best = float("inf")
    for _ in range(iters):
        t0 = time.perf_counter()
        outs = sharded(*dev_in, *dev_zero)
        jax.block_until_ready(outs)
        best = min(best, time.perf_counter() - t0)
    full = np.asarray(outs[out_names.index("out")])
    return best, full


# revision 7
# speedup vs baseline: 1.1301x; 1.0124x over previous
"""Trainium2 Bass kernel v2 for batched differentiable-MPC (LQR) controller.

Riccati backward sweep + forward rollout, B=512 data-parallel over 8 cores
(64 batches/core).

v2 layout ("col-major GRM"): per core, local batch b = 16*g + s with
partition-group g in [0,4) and slot s in [0,16). A per-batch 32x32 z-space
matrix M_b and augmented column q_b live in a [128, 528] fp16 tile:
    tile[32*g + i, 16*c + s] = M_b[i, c]   (c < 32)
    tile[32*g + i, 512 + s]  = q_b[i]      (aug col c=32)
Col-major slots make every column view contiguous and keep the DVE 2x fp16
mode on the broadcast FMA (broadcast over the middle dim, packed last dim).

Backward step: S = Z^T V (PE, fp16) -> J = per-block transpose (DVE, f32
PSUM->SBUF) -> Q = C + Z^T J, qz = q + Z^T v (PE accumulate) -> Act copies
PSUM->SBUF fp16 -> 8-pivot Gauss-Jordan on DVE:
    prow = shuffle (u32-bitcast view: half width)
    m    = col_pc(qa) / col_pc(prow)      (tensor_tensor divide)
    mneg = mask_r - m                     (scalar_tensor_tensor)
    qa  += mneg * prow                    (two fp16 2x tensor_tensor)
Forward: batch-on-partition [64, *] fp16 broadcast-mult + reduce per step.
"""

import os
import sys

import numpy as np

for _p in ("/opt/trn_rl_repo",):
    if _p not in sys.path:
        sys.path.insert(0, _p)

import concourse.bass as bass
import concourse.bacc as bacc
import concourse.mybir as mybir
from concourse import tile
from concourse.bass_utils import run_bass_kernel_spmd

F32 = mybir.dt.float32
F16 = mybir.dt.float16
U32 = mybir.dt.uint32
F32R = mybir.dt.float32r
AX = mybir.AxisListType
OP = mybir.AluOpType

B, T, NX, NU = 512, 100, 24, 8
NZ = NX + NU  # 32
NCORES = 8
BC = B // NCORES  # 64 batches per core
G, SL = 4, 16  # partition groups x free slots
W = NZ + 1  # 33 columns (32 matrix + 1 aug)
FW = SL * W  # 528
MATW = SL * NZ  # 512 (matrix part, contiguous cols 0..512)
XOUT = (T + 1) * NX  # 2424
UOUT = T * NU  # 800
OUT_W = XOUT + UOUT  # 3224

LAST_EXEC_NS = None

_prog_cache = {}


def _build_program(t_steps=T):
    nc = bacc.Bacc("TRN2", target_bir_lowering=False, debug=False)
    Ts = t_steps
    XOUT = (Ts + 1) * NX
    UOUT = Ts * NU
    OUT_W = XOUT + UOUT

    # DRAM I/O (fp16 packed host-side)
    caug = nc.dram_tensor("caug", [Ts, 128, FW], F16, kind="ExternalInput")
    vt0 = nc.dram_tensor("vt0", [128, FW], F16, kind="ExternalInput")
    lz16d = nc.dram_tensor("lz16", [128, 128], F16, kind="ExternalInput")
    id16d = nc.dram_tensor("id16", [128, 128], F16, kind="ExternalInput")
    masksd = nc.dram_tensor("masks", [128, NU * SL], F16, kind="ExternalInput")
    x0p = nc.dram_tensor("x0p", [BC, W], F16, kind="ExternalInput")
    abrep = nc.dram_tensor("abrep", [BC, NX * W], F16, kind="ExternalInput")
    out = nc.dram_tensor("out", [BC, OUT_W], F32, kind="ExternalOutput")
    negmd = nc.dram_tensor("negmask", [128, NU], F16, kind="ExternalInput")
    # gains scratch: final u-rows repacked to (s,c) rows on-chip, so both
    # the store and the forward gather use >=66B-contiguous DMA runs
    kbuf = nc.dram_tensor("kbuf", [G, Ts, NU, W * SL], F16)

    with tile.TileContext(nc) as tc:
        with (
            tc.tile_pool(name="const", bufs=1) as cpool,
            tc.tile_pool(name="cstream", bufs=3) as cs_pool,
            tc.tile_pool(name="qa", bufs=3) as qa_pool,
            tc.tile_pool(name="jt", bufs=3) as j_pool,
            tc.tile_pool(name="prow", bufs=4) as pr_pool,
            tc.tile_pool(name="wide", bufs=4) as wide_pool,
            tc.tile_pool(name="small", bufs=8) as sm_pool,
            tc.tile_pool(name="ps_s", bufs=2, space="PSUM") as ps_s,
            tc.tile_pool(name="ps_q0", bufs=2, space="PSUM") as ps_q0,
            tc.tile_pool(name="ps_q1", bufs=2, space="PSUM") as ps_q1,
            tc.tile_pool(name="ps_v", bufs=2, space="PSUM") as ps_v,
            tc.tile_pool(name="fwd", bufs=1) as f_pool,
            tc.tile_pool(name="kstream", bufs=3) as k_pool,
            tc.tile_pool(name="ftmp", bufs=2) as ft_pool,
        ):
            # ---- constants to SBUF (bounced through DVE for walrus) ----
            def const_tile(name, dram, shape, dt):
                raw = cpool.tile(shape, dt, tag=name + "raw")
                nc.sync.dma_start(out=raw[:], in_=dram[:])
                t_ = cpool.tile(shape, dt, tag=name)
                nc.vector.tensor_copy(out=t_[:], in_=raw[:])
                return t_

            lz16 = const_tile("lz16", lz16d, [128, 128], F16)
            id16 = const_tile("id16", id16d, [128, 128], F16)
            mask_t = cpool.tile([128, NU * SL], F16, tag="masks")
            nc.sync.dma_start(out=mask_t[:], in_=masksd[:])
            negm_t = cpool.tile([128, NU], F16, tag="negmask")
            nc.sync.dma_start(out=negm_t[:], in_=negmd[:])

            # V_T tile
            vcur = const_tile("vterm", vt0, [128, FW], F16)

            def mat_view(t_):  # [128, 512] matrix columns (contiguous)
                return t_[:, 0:MATW]

            def aug_view(t_):  # [128, 16] aug column
                return t_[:, MATW:FW]

            def col_view(t_, c):  # [128, 16] matrix column c
                return t_[:, SL * c : SL * (c + 1)]

            def sc_view(ap):  # free (c,s) -> (s,c) virtual order
                return ap.rearrange("p (c s) -> p s c", s=SL)

            # ---- backward Riccati ----
            for tstep in range(Ts - 1, -1, -1):
                ct = cs_pool.tile([128, FW], F16, tag="ct")
                nc.sync.dma_start(out=ct[:], in_=caug[tstep])

                # The (s,c)-ordered mat pipeline is split into two slot
                # halves (s 0..7 / 8..15 = flat col blocks 0:256 / 256:512)
                # so S -> s16 -> transpose -> Th -> qa-copy pipelines across
                # PE / Act / DVE instead of serializing at full width.
                HH = MATW // 2  # 256

                vq_ps = ps_v.tile([128, SL], F32, tag="vq")
                nc.tensor.matmul(
                    out=vq_ps[:], lhsT=lz16[:], rhs=aug_view(vcur),
                    start=True, stop=False,
                )
                # C lands in PSUM first (off the critical chain: only needs
                # the ct DMA), then Z^T J accumulates on top. Each half is
                # its own accumulation group (own PSUM bank) so its qa copy
                # can start as soon as that half's Th lands.
                ct_sc = sc_view(mat_view(ct))
                q_h = []
                for h, pool in ((0, ps_q0), (1, ps_q1)):
                    qp = pool.tile([128, HH], F32, tag=f"q{h}")
                    q_h.append(qp)
                    nc.tensor.matmul(
                        out=qp[:], lhsT=id16[:],
                        rhs=ct_sc[:, h * 8 : (h + 1) * 8, :],
                        start=True, stop=False,
                    )
                nc.tensor.matmul(
                    out=vq_ps[:], lhsT=id16[:], rhs=aug_view(ct),
                    start=False, stop=True,
                )
                s_ps = ps_s.tile([128, MATW], F32, tag="s")
                s16 = j_pool.tile([128, MATW], F16, tag="s16")
                j_sb = j_pool.tile([128, MATW], F16, tag="j")
                qa = qa_pool.tile([128, FW], F16, tag="qa")
                # aug copy first: vq is ready early and pivot0 needs it
                nc.scalar.copy(out=aug_view(qa), in_=vq_ps[:])
                rhs_sc = sc_view(mat_view(vcur))
                for h in (0, 1):
                    hs = slice(h * HH, (h + 1) * HH)
                    nc.tensor.matmul(
                        out=s_ps[:, hs], lhsT=lz16[:],
                        rhs=rhs_sc[:, h * 8 : (h + 1) * 8, :],
                        start=True, stop=True,
                    )
                    nc.scalar.copy(out=s16[:, hs], in_=s_ps[:, hs])
                    nc.vector.transpose(out=j_sb[:, hs], in_=s16[:, hs])
                    nc.tensor.matmul(
                        out=q_h[h][:], lhsT=lz16[:], rhs=j_sb[:, hs],
                        start=False, stop=True,
                    )
                    # qa mat half: strided (c,s) target, slots h*8..h*8+8
                    qa_half = mat_view(qa).rearrange(
                        "p (c s) -> p s c", s=SL
                    )[:, h * 8 : (h + 1) * 8, :]
                    nc.scalar.copy(out=qa_half, in_=q_h[h][:])

                # ---- 8-pivot Gauss-Jordan on the u block ----
                for r in range(NU):
                    pc = NX + r
                    prow = pr_pool.tile([128, FW], F16, tag="prow")
                    nc.vector.stream_shuffle(
                        out=prow[:].bitcast(U32),
                        in_=qa[:].bitcast(U32),
                        mask=[pc] * 32,
                    )
                    # mneg = (col_pc(qa) * negmask_r) * (1/piv):
                    # negmask_r is -1 per partition, 0 on pivot rows, so the
                    # pivot row is preserved (mneg=0) without a mask add.
                    drec = sm_pool.tile([128, SL], F16, tag="drec")
                    with nc.allow_low_precision(reason="piv >= 1"):
                        nc.vector.reciprocal(
                            out=drec[:], in_=col_view(prow, pc)
                        )
                    mneg = sm_pool.tile([128, SL], F16, tag="mneg")
                    nc.vector.scalar_tensor_tensor(
                        out=mneg[:], in0=col_view(qa, pc),
                        scalar=negm_t[:, r : r + 1],
                        in1=drec[:],
                        op0=OP.mult, op1=OP.mult,
                    )
                    tmp = wide_pool.tile([128, FW], F16, tag="tmp")
                    mneg_b = mneg[:].unsqueeze(1).broadcast_to((128, W, SL))
                    nc.vector.tensor_tensor(
                        out=tmp[:].rearrange("p (c s) -> p c s", s=SL),
                        in0=mneg_b,
                        in1=prow[:].rearrange("p (c s) -> p c s", s=SL),
                        op=OP.mult,
                    )
                    nc.vector.tensor_tensor(
                        out=qa[:], in0=qa[:], in1=tmp[:], op=OP.add
                    )

                # repack gain rows to (s,c) order on the idle Pool engine
                # (off the critical path), then store contiguous rows
                kt_rep = pr_pool.tile([128, FW], F16, tag="ktrep")
                nc.gpsimd.tensor_copy(
                    out=kt_rep[:],
                    in_=qa[:].rearrange("p (c s) -> p s c", s=SL),
                )
                for g in range(G):
                    nc.sync.dma_start(
                        out=kbuf[g, tstep],
                        in_=kt_rep[32 * g + NX : 32 * g + NZ, :],
                    )
                vcur = qa

            # ---- forward rollout (batch on partitions, fp16) ----
            xt = f_pool.tile([BC, W], F16, tag="xt")  # [x | u(=0) | 1]
            nc.sync.dma_start(out=xt[:], in_=x0p[:])
            ab_t = cpool.tile([BC, NX * W], F16, tag="abrep")
            nc.sync.dma_start(out=ab_t[:], in_=abrep[:])
            xall = f_pool.tile([BC, XOUT], F32, tag="xall")
            uall = f_pool.tile([BC, UOUT], F32, tag="uall")

            KQ = 4  # forward steps per kt DMA
            assert Ts % KQ == 0 or Ts < KQ
            kq_eff = KQ if Ts >= KQ else Ts
            kt4 = None
            for tstep in range(Ts):
                j = tstep % kq_eff
                if j == 0:
                    # per group: kq_eff steps of gains in one DMA with
                    # 33-element contiguous runs: kt4[(g,s), ((t',r), c)]
                    kt4 = k_pool.tile([BC, kq_eff * NU * W], F16, tag="kt4")
                    for g in range(G):
                        kb0 = kbuf[g][:]
                        src = bass.AP(
                            tensor=kb0.tensor,
                            offset=kb0.offset + tstep * (NU * W * SL),
                            ap=[
                                [W, SL],                    # s -> partition
                                [W * SL, kq_eff * NU],      # (t', r)
                                [1, W],                     # c contiguous
                            ],
                        )
                        nc.sync.dma_start(
                            out=kt4[g * SL : (g + 1) * SL, :], in_=src
                        )
                kt = kt4[:, j * (NU * W) : (j + 1) * (NU * W)]
                # record x_t (f32 out)
                nc.scalar.copy(
                    out=xall[:, tstep * NX : (tstep + 1) * NX], in_=xt[:, 0:NX]
                )
                # s_r = sum_c kt[r, c] * z[c]   (u slots of z are zero)
                t0 = ft_pool.tile([BC, NU * W], F16, tag="t0")
                nc.vector.tensor_tensor(
                    out=t0[:].rearrange("p (r c) -> p r c", c=W),
                    in0=kt.rearrange("p (r c) -> p r c", c=W),
                    in1=xt[:].unsqueeze(1).broadcast_to((BC, NU, W)),
                    op=OP.mult,
                )
                ssum = ft_pool.tile([BC, NU], F32, tag="ssum")
                nc.vector.tensor_reduce(
                    out=ssum[:],
                    in_=t0[:].rearrange("p (r c) -> p r c", c=W),
                    axis=AX.X, op=OP.add,
                )
                # u = -s / pivot ; pivot at kt[r, 24+r] -> flat 33r + 24 + r
                pivd = bass.AP(
                    tensor=kt.tensor,
                    offset=kt.offset + NX,
                    ap=[list(kt.ap[0]), [W + 1, NU]],
                )
                prec = ft_pool.tile([BC, NU], F32, tag="prec")
                nc.vector.reciprocal(out=prec[:], in_=pivd)
                ut = ft_pool.tile([BC, NU], F16, tag="ut")
                nc.vector.scalar_tensor_tensor(
                    out=ut[:], in0=ssum[:], scalar=-1.0, in1=prec[:],
                    op0=OP.mult, op1=OP.mult,
                )
                nc.scalar.copy(
                    out=uall[:, tstep * NU : (tstep + 1) * NU], in_=ut[:]
                )
                # z = [x | u | 1]
                nc.vector.tensor_copy(out=xt[:, NX:NZ], in_=ut[:])
                # x' = [A B 0] z
                t2 = ft_pool.tile([BC, NX * W], F16, tag="t2")
                nc.vector.tensor_tensor(
                    out=t2[:].rearrange("p (i c) -> p i c", c=W),
                    in0=ab_t[:].rearrange("p (i c) -> p i c", c=W),
                    in1=xt[:].unsqueeze(1).broadcast_to((BC, NX, W)),
                    op=OP.mult,
                )
                with nc.allow_low_precision(reason="fp16 state"):
                    nc.vector.tensor_reduce(
                        out=xt[:, 0:NX],
                        in_=t2[:].rearrange("p (i c) -> p i c", c=W),
                        axis=AX.X, op=OP.add,
                    )
                # restore z invariant [x | 0 | 1] for the next step's s_r
                nc.vector.memset(xt[:, NX:NZ], 0.0)
            nc.scalar.copy(out=xall[:, Ts * NX : (Ts + 1) * NX], in_=xt[:, 0:NX])
            nc.sync.dma_start(out=out[:, 0:XOUT], in_=xall[:])
            nc.sync.dma_start(out=out[:, XOUT:OUT_W], in_=uall[:])

    nc.compile()
    return nc


def _host_pack(inputs, t_steps=T):
    """q precompute + per-core col-major GRM packing (fp16). Returns in_maps."""
    Ts = t_steps
    x0 = np.asarray(inputs["x0"], np.float32)
    C = np.asarray(inputs["C"], np.float32)[:, :Ts]
    c = np.asarray(inputs["c"], np.float32)[:, :Ts]
    C_final = np.asarray(inputs["C_final"], np.float32)
    c_final = np.asarray(inputs["c_final"], np.float32)
    x_ref = np.asarray(inputs["x_ref"], np.float32)
    u_ref = np.asarray(inputs["u_ref"], np.float32)[:, :Ts]
    A = np.asarray(inputs["A_dyn"], np.float32)
    Bd = np.asarray(inputs["B_dyn"], np.float32)

    xr = x_ref[:, : Ts + 1] if x_ref.shape[1] > Ts else x_ref
    zref = np.concatenate([xr[:, :Ts], u_ref], axis=-1)  # [B,Ts,32]
    q = c - np.einsum("btij,btj->bti", C, zref)
    VT = C_final[:, :NX, :NX]
    vT = c_final[:, :NX] - np.einsum("bij,bj->bi", VT, xr[:, Ts])

    # caug [cores, Ts, G, 32(i), 33(c), SL(s)]
    caug = np.zeros((NCORES, Ts, G, NZ, W, SL), np.float32)
    Cb = C.reshape(NCORES, G, SL, Ts, NZ, NZ)
    caug[..., 0:NZ, :] = Cb.transpose(0, 3, 1, 4, 5, 2)
    qb = q.reshape(NCORES, G, SL, Ts, NZ)
    caug[..., NZ, :] = qb.transpose(0, 3, 1, 4, 2)
    caug = np.ascontiguousarray(
        caug.reshape(NCORES, Ts, 128, FW)
    ).astype(np.float16)

    vt0 = np.zeros((NCORES, G, NZ, W, SL), np.float32)
    VTb = VT.reshape(NCORES, G, SL, NX, NX)
    vt0[:, :, 0:NX, 0:NX, :] = VTb.transpose(0, 1, 3, 4, 2)
    vTb = vT.reshape(NCORES, G, SL, NX)
    vt0[:, :, 0:NX, NZ, :] = vTb.transpose(0, 1, 3, 2)
    vt0 = np.ascontiguousarray(vt0.reshape(NCORES, 128, FW)).astype(np.float16)

    AB = np.concatenate([A, Bd], axis=1)  # [24, 32]
    Zpad = np.zeros((NZ, NZ), np.float32)
    Zpad[0:NX, :] = AB
    lz = np.zeros((128, 128), np.float32)
    for g in range(G):
        lz[32 * g : 32 * g + NZ, 32 * g : 32 * g + NZ] = Zpad
    lz16 = lz.astype(np.float16)
    id16 = np.eye(128, dtype=np.float16)

    masks = np.zeros((128, NU * SL), np.float16)
    for r in range(NU):
        for g in range(G):
            masks[32 * g + NX + r, r * SL : (r + 1) * SL] = 1.0
    negmask = np.full((128, NU), -1.0, np.float16)
    for r in range(NU):
        for g in range(G):
            negmask[32 * g + NX + r, r] = 0.0

    x0p = np.zeros((NCORES, BC, W), np.float32)
    x0b = x0.reshape(NCORES, G, SL, NX)
    for g in range(G):
        x0p[:, g * SL : (g + 1) * SL, 0:NX] = x0b[:, g]
    x0p[:, :, NZ] = 1.0
    x0p = x0p.astype(np.float16)

    abaug = np.zeros((NX, W), np.float32)
    abaug[:, 0:NZ] = AB
    abrep = np.ascontiguousarray(
        np.broadcast_to(abaug.reshape(1, NX * W), (BC, NX * W))
    ).astype(np.float16)

    in_maps = []
    for core in range(NCORES):
        in_maps.append(
            {
                "caug": caug[core],
                "vt0": vt0[core],
                "lz16": lz16,
                "id16": id16,
                "masks": masks,
                "negmask": negmask,
                "x0p": x0p[core],
                "abrep": abrep[core] if abrep.ndim == 3 else abrep,
            }
        )
    return in_maps


def _unpack(results):
    outs = [results[core]["out"] for core in range(NCORES)]
    return np.concatenate(outs, axis=0)


def kernel(**inputs):
    global LAST_EXEC_NS
    trace = bool(int(os.environ.get("KERNEL_TRACE", "0")))
    key = ("prog", trace)
    if key not in _prog_cache:
        _prog_cache[key] = _build_program()
    nc = _prog_cache[key]
    in_maps = _host_pack(inputs)
    res = run_bass_kernel_spmd(
        nc, in_maps, core_ids=list(range(NCORES)), trace=trace
    )
    LAST_EXEC_NS = res.exec_time_ns
    return _unpack(res.results)


def bench(inputs, iters=10):
    """Device-resident repeated execution timing (same path as baseline)."""
    import time

    import jax
    from jax.sharding import Mesh, NamedSharding, PartitionSpec
    from jax.experimental.shard_map import shard_map

    from concourse import bass2jax as B2J

    key = ("prog", False)
    if key not in _prog_cache:
        _prog_cache[key] = _build_program()
    nc = _prog_cache[key]
    in_maps = _host_pack(inputs)

    B2J.install_neuronx_cc_hook()
    in_names, out_names, out_avals, zero_outs = [], [], [], []
    for alloc in nc.m.functions[0].allocations:
        if not isinstance(alloc, mybir.MemoryLocationSet):
            continue
        name = alloc.memorylocations[0].name
        if alloc.kind == "ExternalInput":
            if (
                nc.partition_id_tensor is not None
                and name == nc.partition_id_tensor.name
            ):
                continue
            in_names.append(name)
        elif alloc.kind == "ExternalOutput":
            out_names.append(name)
            shape = tuple(alloc.tensor_shape)
            dtype = mybir.dt.np(alloc.dtype)
            out_avals.append(jax.core.ShapedArray(shape, dtype))
            zero_outs.append(np.zeros(shape, dtype))
    n_params = len(in_names)
    all_in_names = list(in_names) + list(out_names)
    partition_name = (
        nc.partition_id_tensor.name if nc.partition_id_tensor else None
    )
    if partition_name is not None:
        all_in_names.append(partition_name)

    def _body(*args):
        operands = list(args)
        if partition_name is not None:
            operands.append(B2J.partition_id_tensor())
        outs = B2J._bass_exec_p.bind(
            *operands,
            out_avals=tuple(out_avals),
            in_names=tuple(all_in_names),
            out_names=tuple(out_names),
            lowering_input_output_aliases=(),
            sim_require_finite=True,
            sim_require_nnan=True,
            nc=nc,
        )
        return tuple(outs)

    devices = jax.devices()[:NCORES]
    mesh = Mesh(np.asarray(devices), ("core",))
    nops = n_params + len(out_names)
    sharded = jax.jit(
        shard_map(
            _body,
            mesh=mesh,
            in_specs=(PartitionSpec("core"),) * nops,
            out_specs=(PartitionSpec("core"),) * len(out_names),
            check_rep=False,
        ),
        keep_unused=True,
    )
    sh = NamedSharding(mesh, PartitionSpec("core"))
    dev_in = [
        jax.device_put(
            np.concatenate(
                [np.asarray(in_maps[c][n]) for c in range(NCORES)], axis=0
            ),
            sh,
        )
        for n in in_names
    ]
    dev_zero = [
        jax.device_put(
            np.zeros((NCORES * z.shape[0], *z.shape[1:]), z.dtype), sh
        )
        for z in zero_outs
    ]
    outs = sharded(*dev_in, *dev_zero)
    jax.block_until_ready(outs)
    best = float("inf")
    for _ in range(iters):
        t0 = time.perf_counter()
        outs = sharded(*dev_in, *dev_zero)
        jax.block_until_ready(outs)
        best = min(best, time.perf_counter() - t0)
    full = np.asarray(outs[out_names.index("out")])
    return best, full
